# revision 27
# baseline (speedup 1.0000x reference)
"""DiffusionConv (4x GCN message passing) Trainium2 kernel, 8-core SPMD.

Strategy: shard destination nodes across 8 cores (3750 each). x is sharded
by node across cores (2.9MB/core instead of a 23MB replica) and re-assembled
on device with an AllGather collective into a local DRAM scratch copy per
core. Each core then runs a fused window-major pipeline; per 128-dst window:
  - gathers source-node feature rows (fp16) for its edges via dma_gather
    (4 SWDGE queues round-robin), edges pre-sorted by destination and
    padded per 128-dst window,
  - builds the banded [128e x 128d] edge-weight matrices on-chip (DVE
    iota-compare), aggregates with one matmul per 128-edge chunk
    accumulating in PSUM,
  - applies the 32x32 weight matrices via PE transpose + block-diagonal
    matmuls contracting over (adjacency, channel), adds bias, and writes
    the window's fp16 output rows (host upconverts to f32).
Host<->device traffic is minimized (33MB up / 23MB down vs 284MB total for
the replicated-x baseline): gather indices ship de-replicated ([16, n]
int16, expanded 8x on device), per-edge dst columns ship uint8 and are cast
to fp16 during the SWDGE DMA, and the block-diagonal W tiles plus all
iota/bias constants are constructed on device from the raw [32,32] weights.
"""
import sys, os
for p in ('/opt/trn_rl_repo', '/root/.axon_site/_ro/trn_rl_repo'):
    if os.path.isdir(p) and p not in sys.path:
        sys.path.insert(0, p)

import numpy as np
import ml_dtypes

N = 30000
C = 32
T = 12
ES = C * T          # 384, feature row width
E = 480000
NCORES = 8
ND = N // NCORES    # 3750 dst nodes per core
NDP = 3840          # padded to 30 windows of 128
WIN = 128           # dst window width
NWIN = NDP // WIN   # 30
ADJS = ("fwd1", "fwd2", "bck1", "bck2")

bf16 = np.float16 if os.environ.get("K_DT", "fp16") == "fp16" else ml_dtypes.bfloat16


def _prep_edges(ei, ew):
    """Split one adjacency's edges by destination core and sort by dst."""
    src = np.asarray(ei[0]).astype(np.int64)
    dst = np.asarray(ei[1]).astype(np.int64)
    w = np.asarray(ew).astype(np.float32)
    core = dst // ND
    out = []
    for k in range(NCORES):
        sel = core == k
        dl = dst[sel] - k * ND
        s = src[sel]
        wv = w[sel]
        order = np.argsort(dl, kind="stable")
        dl = dl[order]
        s = s[order]
        wv = wv[order]
        win = dl // WIN
        bounds = np.searchsorted(win, np.arange(NWIN + 1))
        counts = np.diff(bounds)
        out.append((s, dl, wv, bounds, counts))
    return out


def _build_adj_inputs(per_core):
    """Uniform-shape device inputs for one adjacency: gather idx tiles and
    per-edge (dst-local column, weight) tiles; per-window chunk counts are
    shared across cores (max)."""
    ncw = np.maximum(1, -(-np.stack([pc[4] for pc in per_core], 0).max(0) // 128))
    nch = int(ncw.sum())
    epad = nch * 128
    winid = np.repeat(np.arange(NWIN), ncw * 128)
    gis, dlcs = [], []
    for k in range(NCORES):
        s, dl, wv, bounds, counts = per_core[k]
        srcp = np.zeros(epad, np.int64)
        dlp = np.zeros(epad, np.int64)
        ewp = np.zeros(epad, np.float32)
        off = 0
        for w in range(NWIN):
            c = int(counts[w])
            lo, hi = int(bounds[w]), int(bounds[w + 1])
            srcp[off:off + c] = s[lo:hi]
            dlp[off:off + c] = dl[lo:hi]
            ewp[off:off + c] = wv[lo:hi]
            dlp[off + c:off + int(ncw[w]) * 128] = w * WIN  # pads (ew 0)
            off += int(ncw[w]) * 128
        # gather idx, wrapped layout [16, epad//16]: idx i -> [i%16, i//16]
        # (replicated 8x across partition groups on device)
        gi = srcp.astype(np.int16).reshape(-1, 16).T
        col = dlp - winid * WIN                        # 0..127
        # device layout [128 partitions = edge-in-chunk, nch]
        dlc = col.reshape(nch, 128).T.astype(np.uint8)
        ewc = ewp.reshape(nch, 128).T.astype(bf16)
        gis.append(np.ascontiguousarray(gi))
        dlcs.append((np.ascontiguousarray(dlc), np.ascontiguousarray(ewc)))
    return ncw.astype(np.int64), gis, dlcs


_CACHE = {}
LAST_RESULTS = None
LAST_NC = None
LAST_NCWS = None
LAST_INMAPS = None


def _get_program(ncws, rep=1, ablate=()):
    """Build (and cache) the Bass program for given per-adjacency window
    chunk counts. ncws: tuple of 4 tuples of NWIN ints. rep>1 repeats the
    pipeline (device-time measurement via slope); ablate disables stages
    for profiling ("nogather", "noaggmm", "nowphase", "nostbuild", "noag").
    """
    key = (ncws, rep, tuple(sorted(ablate)))
    if key in _CACHE:
        return _CACHE[key]

    import concourse.mybir as mybir
    import concourse.tile as tile
    from concourse import bacc
    from concourse.masks import make_identity

    DT = (mybir.dt.float16 if os.environ.get("K_DT", "fp16") == "fp16"
          else mybir.dt.bfloat16)
    nchs = [int(sum(ncw)) for ncw in ncws]

    nc = bacc.Bacc("TRN2", target_bir_lowering=False, debug=False,
                   num_devices=NCORES, num_swdge_queues=4)
    xsh_d = nc.dram_tensor("xsh", [ND, ES], DT, kind="ExternalInput")
    gi_d = [nc.dram_tensor(f"gi_{a}", [16, nchs[ai] * 8], mybir.dt.int16,
                           kind="ExternalInput") for ai, a in enumerate(ADJS)]
    dlc_d = [nc.dram_tensor(f"dlc_{a}", [128, nchs[ai]], mybir.dt.uint8,
                            kind="ExternalInput") for ai, a in enumerate(ADJS)]
    ew_d = [nc.dram_tensor(f"ew_{a}", [128, nchs[ai]], DT,
                           kind="ExternalInput") for ai, a in enumerate(ADJS)]
    W_d = nc.dram_tensor("Wmats", [32, 4, 32], DT, kind="ExternalInput")
    bias_d = nc.dram_tensor("biasrow", [1, ES], mybir.dt.float32, kind="ExternalInput")
    crow_d = nc.dram_tensor("crows", [1, 128], DT, kind="ExternalInput")
    out_d = nc.dram_tensor("out", [ND, ES], DT, kind="ExternalOutput")

    gq = [0]  # gather queue rotation counter
    GBLK = int(os.environ.get('K_GBLK', 8))   # chunks per gather block
    GBUFS = int(os.environ.get('K_GBUFS', 10))  # msg buffers in flight

    with tile.TileContext(nc) as tc:
        with tc.tile_pool(name="const", bufs=1) as cpool, \
             tc.tile_pool(name="dram", bufs=1, space="DRAM") as dpool, \
             tc.tile_pool(name="ybuf", bufs=1) as ypool, \
             tc.tile_pool(name="work", bufs=3) as wpool, \
             tc.tile_pool(name="wphase", bufs=2) as opool, \
             tc.tile_pool(name="yT", bufs=13) as ytpool, \
             tc.tile_pool(name="psagg", bufs=4, space="PSUM") as ps_agg, \
             tc.tile_pool(name="pstp", bufs=2, space="PSUM") as ps_tp, \
             tc.tile_pool(name="psout", bufs=2, space="PSUM") as ps_out:

            # ---- on-device constants ----
            crow_t = cpool.tile([1, 128], DT)
            nc.sync.dma_start(out=crow_t[:], in_=crow_d.ap())
            bias_t = cpool.tile([1, ES], mybir.dt.float32)
            nc.sync.dma_start(out=bias_t[:], in_=bias_d.ap())
            Wsb = cpool.tile([32, 4, 32], DT)
            nc.sync.dma_start(out=Wsb[:], in_=W_d.ap())
            ident = cpool.tile([128, 128], DT)
            make_identity(nc, ident[:])
            ones_t = cpool.tile([1, 128], DT)
            nc.vector.memset(ones_t[:], 1.0)
            onesf_t = cpool.tile([1, 128], mybir.dt.float32)
            nc.vector.memset(onesf_t[:], 1.0)

            # iota row broadcast to all 128 partitions via rank-1 matmul:
            # iota_t[p, j] = j
            iota_t = cpool.tile([128, 128], DT)
            psI = ps_tp.tile([128, 128], mybir.dt.float32, tag="tp")
            nc.tensor.matmul(psI[:], lhsT=ones_t[:], rhs=crow_t[:],
                             start=True, stop=True)
            nc.scalar.copy(out=iota_t[:], in_=psI[:])

            # ET[c, p] = (p % 32 == c): four side-by-side 32x32 identities
            ET = cpool.tile([32, 128], DT)
            for j in range(4):
                nc.vector.tensor_copy(out=ET[:, 32 * j:32 * (j + 1)],
                                      in_=ident[:32, :32])

            # block-diagonal W tiles B[a][k]: [128, 384]:
            # B[a,k][p, c'*12 + t] = W_a[p % 32, c'] with t = 4k + p//32
            B_t = cpool.tile([128, 4, 3, ES], DT)
            nc.vector.memset(B_t[:], 0.0)
            for a in range(4):
                psG = ps_tp.tile([128, 32], mybir.dt.float32, tag="tp")
                nc.tensor.matmul(psG[:], lhsT=ET[:], rhs=Wsb[:, a, :],
                                 start=True, stop=True)
                G = cpool.tile([128, 32], DT, name=f"G{a}")
                nc.scalar.copy(out=G[:], in_=psG[:])
                for k in range(3):
                    Bv = B_t[:, a, k, :].rearrange("p (c t) -> p t c", t=T)
                    for j in range(4):
                        nc.vector.tensor_copy(
                            out=Bv[32 * j:32 * (j + 1), 4 * k + j, :],
                            in_=G[32 * j:32 * (j + 1), :])

            # bias broadcast to 128 partitions via fp32 rank-1 matmul
            biasb_t = cpool.tile([128, ES], mybir.dt.float32)
            psB = ps_out.tile([128, ES], mybir.dt.float32, tag="wout")
            nc.tensor.matmul(psB[:], lhsT=onesf_t[:], rhs=bias_t[:],
                             start=True, stop=True)
            nc.scalar.copy(out=biasb_t[:], in_=psB[:])

            max_nch = max(int(v) for ncw in ncws for v in ncw)
            choffs = [[int(sum(ncws[ai][:w])) for w in range(NWIN)]
                      for ai in range(4)]

            for _rep in range(rep):
                # ---- x all-gather: 3750-row shard -> full 30000-row copy ----
                xint = dpool.tile([ND, ES], DT, tag="xint", bufs=2)
                xfull = dpool.tile([N, ES], DT, tag="xfull", bufs=2)
                nc.gpsimd.dma_start(out=xint[:], in_=xsh_d.ap())
                if "noag" in ablate:
                    nc.gpsimd.dma_start(out=xfull[:ND], in_=xint[:])
                else:
                    nc.gpsimd.collective_compute(
                        "AllGather", mybir.AluOpType.bypass,
                        replica_groups=[list(range(NCORES))],
                        ins=[xint[:].opt()], outs=[xfull[:].opt()])

                # ---- resident edge tensors, loaded once per pass ----
                # gather idx: the [16, n] host tensor replicated into all 8
                # partition groups; dst columns: uint8, cast to DT in-DMA
                idx_all = [wpool.tile([128, nchs[ai] * 8], mybir.dt.int16,
                                      tag=f"idx{a}", bufs=1, name=f"idx_{a}")
                           for ai, a in enumerate(ADJS)]
                dlc_all = [wpool.tile([128, nchs[ai]], DT,
                                      tag=f"dlc{a}", bufs=1, name=f"dlc_{a}")
                           for ai, a in enumerate(ADJS)]
                ew_all = [wpool.tile([128, nchs[ai]], DT,
                                     tag=f"ew{a}", bufs=1, name=f"ewt_{a}")
                          for ai, a in enumerate(ADJS)]
                for ai in range(4):
                    for r in range(8):
                        nc.sync.dma_start(out=idx_all[ai][16 * r:16 * (r + 1), :],
                                          in_=gi_d[ai].ap())
                    nc.gpsimd.dma_start(out=dlc_all[ai][:], in_=dlc_d[ai].ap())
                    nc.scalar.dma_start(out=ew_all[ai][:], in_=ew_d[ai].ap())

                # ---- fused aggregation + W transform, window-major ----
                for w in range(NWIN):
                    yws = []
                    for ai in range(4):
                        nch = int(ncws[ai][w])
                        choff = choffs[ai][w]
                        # on-chip banded matrix build (whole window)
                        st_t = wpool.tile([128, max_nch, WIN], DT, tag="st", bufs=2)
                        dlc_t = dlc_all[ai][:, choff:choff + nch]
                        ewc_t = ew_all[ai][:, choff:choff + nch]
                        idx_t = idx_all[ai]
                        if "nostbuild" in ablate:
                            nc.vector.tensor_copy(
                                out=st_t[:, :nch, :],
                                in_=iota_t[:].rearrange("p (o i) -> p o i", o=1)
                                    .to_broadcast([128, nch, WIN]))
                        else:
                            dlc_b = dlc_t.rearrange("p (n o) -> p n o", o=1) \
                                .to_broadcast([128, nch, WIN])
                            iota_b = iota_t[:].rearrange("p (o i) -> p o i", o=1) \
                                .to_broadcast([128, nch, WIN])
                            ewc_b = ewc_t.rearrange("p (n o) -> p n o", o=1) \
                                .to_broadcast([128, nch, WIN])
                            nc.vector.tensor_tensor(out=st_t[:, :nch, :], in0=dlc_b,
                                                    in1=iota_b,
                                                    op=mybir.AluOpType.is_equal)
                            nc.vector.tensor_tensor(out=st_t[:, :nch, :],
                                                    in0=st_t[:, :nch, :], in1=ewc_b,
                                                    op=mybir.AluOpType.mult)
                        ps = ps_agg.tile([128, ES], mybir.dt.float32, tag="agg")
                        # gather in fixed-size blocks for deep DMA pipelining
                        for b0 in range(0, nch, GBLK):
                            bn = min(GBLK, nch - b0)
                            msg = wpool.tile([128, GBLK, ES], DT,
                                             tag="msg", bufs=GBUFS)
                            if "nogather" not in ablate:
                                nc.gpsimd.dma_gather(
                                    msg[:, :bn, :], xfull[:].opt(),
                                    idx_t[:, (choff + b0) * 8:(choff + b0 + bn) * 8],
                                    bn * 128, bn * 128, ES, elem_step=ES,
                                    single_packet=False, queue_num=gq[0] % 4)
                                gq[0] += 1
                            else:
                                nc.gpsimd.memset(msg[:], 0.0)
                            if "noaggmm" in ablate:
                                nc.tensor.matmul(ps[:, :], lhsT=st_t[:, b0, :],
                                                 rhs=msg[:, 0, :],
                                                 start=(b0 == 0), stop=(b0 + bn >= nch))
                            else:
                                for j in range(bn):
                                    ch = b0 + j
                                    nc.tensor.matmul(ps[:, :], lhsT=st_t[:, ch, :],
                                                     rhs=msg[:, j, :],
                                                     start=(ch == 0), stop=(ch == nch - 1))
                        yw = ypool.tile([128, ES], DT, tag="yw", bufs=8,
                                        name=f"yw_{ai}")
                        nc.scalar.copy(out=yw[:], in_=ps[:])
                        yws.append(yw)

                    if "nowphase" in ablate:
                        continue
                    yTs = []
                    for ai in range(4):
                        for k in range(3):
                            pst = ps_tp.tile([128, 128], DT, tag="tp")
                            nc.tensor.transpose(
                                pst[:], yws[ai][:, 128 * k:128 * (k + 1)], ident[:])
                            yT = ytpool.tile([128, 128], DT, tag="yT")
                            eng = nc.vector if (ai * 3 + k) % 2 == 0 else nc.scalar
                            if eng is nc.vector:
                                eng.tensor_copy(out=yT[:], in_=pst[:])
                            else:
                                eng.copy(out=yT[:], in_=pst[:])
                            yTs.append(yT)
                    pso = ps_out.tile([128, ES], mybir.dt.float32, tag="wout")
                    for i, yT in enumerate(yTs):
                        ai, k = divmod(i, 3)
                        nc.tensor.matmul(pso[:], lhsT=yT[:], rhs=B_t[:, ai, k, :],
                                         start=(i == 0), stop=(i == 11))
                    outsb = opool.tile([128, ES], DT, tag="out")
                    nc.vector.tensor_tensor(out=outsb[:], in0=pso[:], in1=biasb_t[:],
                                            op=mybir.AluOpType.add)
                    nr = min(128, ND - 128 * w)
                    nc.sync.dma_start(out=out_d.ap()[128 * w:128 * w + nr, :],
                                      in_=outsb[:nr])

    nc.compile()
    _CACHE[key] = nc
    return nc


def _host_prep(x, Ws, bias, eis, ews):
    x_rows = np.ascontiguousarray(
        np.asarray(x).astype(np.float32).transpose(0, 2, 1).reshape(N, ES)).astype(bf16)
    ncws, gis, dlcs = [], [], []
    for a in ADJS:
        pc = _prep_edges(np.asarray(eis[a]), np.asarray(ews[a]))
        ncw, gi, dew = _build_adj_inputs(pc)
        ncws.append(tuple(int(v) for v in ncw))
        gis.append(gi)
        dlcs.append(dew)
    Wmats = np.ascontiguousarray(
        np.stack([np.asarray(w).astype(np.float32) for w in Ws], 1)).astype(bf16)
    biasrow = np.ascontiguousarray(
        np.repeat(np.asarray(bias).astype(np.float32), T)[None, :])
    crows = np.ascontiguousarray(
        np.arange(128, dtype=np.float32)[None, :]).astype(bf16)
    in_maps = []
    for k in range(NCORES):
        m = {"xsh": np.ascontiguousarray(x_rows[k * ND:(k + 1) * ND]),
             "Wmats": Wmats, "biasrow": biasrow, "crows": crows}
        for ai, a in enumerate(ADJS):
            m[f"gi_{a}"] = gis[ai][k]
            m[f"dlc_{a}"] = dlcs[ai][k][0]
            m[f"ew_{a}"] = dlcs[ai][k][1]
        in_maps.append(m)
    return tuple(ncws), in_maps


def kernel(x, W_fwd1, W_fwd2, W_bck1, W_bck2, bias,
           ew_fwd1, ew_fwd2, ew_bck1, ew_bck2,
           ei_fwd1, ei_fwd2, ei_bck1, ei_bck2):
    from concourse.bass_utils import run_bass_kernel_spmd

    x = np.asarray(x)
    eis = dict(fwd1=ei_fwd1, fwd2=ei_fwd2, bck1=ei_bck1, bck2=ei_bck2)
    ews = dict(fwd1=ew_fwd1, fwd2=ew_fwd2, bck1=ew_bck1, bck2=ew_bck2)
    Ws = [W_fwd1, W_fwd2, W_bck1, W_bck2]

    ncws, in_maps = _host_prep(x, Ws, bias, eis, ews)
    nc = _get_program(ncws)

    res = run_bass_kernel_spmd(nc, in_maps, core_ids=list(range(NCORES)))
    global LAST_RESULTS, LAST_NC, LAST_INMAPS, LAST_NCWS
    LAST_RESULTS = res
    LAST_NC = nc
    LAST_INMAPS = in_maps
    LAST_NCWS = ncws

    out = np.empty((N, C, T), np.float32)
    for k in range(NCORES):
        shard = res.results[k]["out"]                # [3750, 384], phi'=c*12+t
        out[k * ND:(k + 1) * ND] = shard.astype(np.float32).reshape(ND, C, T)
    return out


# revision 29
# speedup vs baseline: 1.3030x; 1.3030x over previous
"""DiffusionConv (4x GCN message passing) Trainium2 kernel, 8-core SPMD.

Strategy: shard destination nodes across 8 cores (3750 each). x is sharded
by node across cores (2.9MB/core instead of a 23MB replica) and re-assembled
on device with an AllGather collective into a local DRAM scratch copy per
core. Each core then runs a fused window-major pipeline; per 128-dst window:
  - gathers source-node feature rows (fp16) for its edges via dma_gather
    (4 SWDGE queues round-robin), edges pre-sorted by destination and
    padded per 128-dst window,
  - builds the banded [128e x 128d] edge-weight matrices on-chip (DVE
    iota-compare), aggregates with one matmul per 128-edge chunk
    accumulating in PSUM,
  - applies the 32x32 weight matrices via PE transpose + block-diagonal
    matmuls contracting over (adjacency, channel), adds bias, and writes
    the window's fp16 output rows (host upconverts to f32).
Host<->device traffic is minimized (33MB up / 23MB down vs 284MB total for
the replicated-x baseline): gather indices ship de-replicated ([16, n]
int16, expanded 8x on device), per-edge dst columns ship uint8 and are cast
to fp16 during the SWDGE DMA, and the block-diagonal W tiles plus all
iota/bias constants are constructed on device from the raw [32,32] weights.
"""
import sys, os
for p in ('/opt/trn_rl_repo', '/root/.axon_site/_ro/trn_rl_repo'):
    if os.path.isdir(p) and p not in sys.path:
        sys.path.insert(0, p)

import numpy as np
import ml_dtypes

N = 30000
C = 32
T = 12
ES = C * T          # 384, feature row width
E = 480000
NCORES = 8
ND = N // NCORES    # 3750 dst nodes per core
NDP = 3840          # padded to 30 windows of 128
WIN = 128           # dst window width
NWIN = NDP // WIN   # 30
ADJS = ("fwd1", "fwd2", "bck1", "bck2")

bf16 = np.float16 if os.environ.get("K_DT", "fp16") == "fp16" else ml_dtypes.bfloat16


def _prep_edges(ei, ew):
    """Split one adjacency's edges by destination core and sort by dst."""
    src = np.asarray(ei[0]).astype(np.int64)
    dst = np.asarray(ei[1]).astype(np.int64)
    w = np.asarray(ew).astype(np.float32)
    core = dst // ND
    out = []
    for k in range(NCORES):
        sel = core == k
        dl = dst[sel] - k * ND
        s = src[sel]
        wv = w[sel]
        order = np.argsort(dl, kind="stable")
        dl = dl[order]
        s = s[order]
        wv = wv[order]
        win = dl // WIN
        bounds = np.searchsorted(win, np.arange(NWIN + 1))
        counts = np.diff(bounds)
        out.append((s, dl, wv, bounds, counts))
    return out


def _build_adj_inputs(per_core):
    """Uniform-shape device inputs for one adjacency: gather idx tiles and
    per-edge (dst-local column, weight) tiles; per-window chunk counts are
    shared across cores (max)."""
    ncw = np.maximum(1, -(-np.stack([pc[4] for pc in per_core], 0).max(0) // 128))
    nch = int(ncw.sum())
    epad = nch * 128
    winid = np.repeat(np.arange(NWIN), ncw * 128)
    gis, dlcs = [], []
    for k in range(NCORES):
        s, dl, wv, bounds, counts = per_core[k]
        srcp = np.zeros(epad, np.int64)
        dlp = np.zeros(epad, np.int64)
        ewp = np.zeros(epad, np.float32)
        off = 0
        for w in range(NWIN):
            c = int(counts[w])
            lo, hi = int(bounds[w]), int(bounds[w + 1])
            srcp[off:off + c] = s[lo:hi]
            dlp[off:off + c] = dl[lo:hi]
            ewp[off:off + c] = wv[lo:hi]
            dlp[off + c:off + int(ncw[w]) * 128] = w * WIN  # pads (ew 0)
            off += int(ncw[w]) * 128
        # gather idx, wrapped layout [16, epad//16]: idx i -> [i%16, i//16]
        # (replicated 8x across partition groups on device)
        gi = srcp.astype(np.int16).reshape(-1, 16).T
        col = dlp - winid * WIN                        # 0..127
        # device layout [128 partitions = edge-in-chunk, nch]
        dlc = col.reshape(nch, 128).T.astype(np.uint8)
        ewc = ewp.reshape(nch, 128).T.astype(bf16)
        gis.append(np.ascontiguousarray(gi))
        dlcs.append((np.ascontiguousarray(dlc), np.ascontiguousarray(ewc)))
    return ncw.astype(np.int64), gis, dlcs


_CACHE = {}
LAST_RESULTS = None
LAST_NC = None
LAST_NCWS = None
LAST_INMAPS = None


def _get_program(ncws, rep=1, ablate=()):
    """Build (and cache) the Bass program for given per-adjacency window
    chunk counts. ncws: tuple of 4 tuples of NWIN ints. rep>1 repeats the
    pipeline (device-time measurement via slope); ablate disables stages
    for profiling ("nogather", "noaggmm", "nowphase", "nostbuild", "noag").
    """
    key = (ncws, rep, tuple(sorted(ablate)))
    if key in _CACHE:
        return _CACHE[key]

    import concourse.mybir as mybir
    import concourse.tile as tile
    from concourse import bacc
    from concourse.masks import make_identity

    DT = (mybir.dt.float16 if os.environ.get("K_DT", "fp16") == "fp16"
          else mybir.dt.bfloat16)
    nchs = [int(sum(ncw)) for ncw in ncws]

    nc = bacc.Bacc("TRN2", target_bir_lowering=False, debug=False,
                   num_devices=NCORES, num_swdge_queues=4)
    xsh_d = nc.dram_tensor("xsh", [ND, ES], DT, kind="ExternalInput")
    gi_d = [nc.dram_tensor(f"gi_{a}", [16, nchs[ai] * 8], mybir.dt.int16,
                           kind="ExternalInput") for ai, a in enumerate(ADJS)]
    dlc_d = [nc.dram_tensor(f"dlc_{a}", [128, nchs[ai]], mybir.dt.uint8,
                            kind="ExternalInput") for ai, a in enumerate(ADJS)]
    ew_d = [nc.dram_tensor(f"ew_{a}", [128, nchs[ai]], DT,
                           kind="ExternalInput") for ai, a in enumerate(ADJS)]
    W_d = nc.dram_tensor("Wmats", [32, 4, 32], DT, kind="ExternalInput")
    bias_d = nc.dram_tensor("biasrow", [1, ES], mybir.dt.float32, kind="ExternalInput")
    crow_d = nc.dram_tensor("crows", [1, 128], DT, kind="ExternalInput")
    out_d = nc.dram_tensor("out", [ND, ES], DT, kind="ExternalOutput")

    gq = [0]  # gather queue rotation counter
    GBLK = int(os.environ.get('K_GBLK', 10))  # chunks per gather block
    GBUFS = int(os.environ.get('K_GBUFS', 8))  # msg buffers in flight

    with tile.TileContext(nc) as tc:
        with tc.tile_pool(name="const", bufs=1) as cpool, \
             tc.tile_pool(name="dram", bufs=1, space="DRAM") as dpool, \
             tc.tile_pool(name="ybuf", bufs=1) as ypool, \
             tc.tile_pool(name="work", bufs=3) as wpool, \
             tc.tile_pool(name="wphase", bufs=2) as opool, \
             tc.tile_pool(name="yT", bufs=13) as ytpool, \
             tc.tile_pool(name="psagg", bufs=4, space="PSUM") as ps_agg, \
             tc.tile_pool(name="pstp", bufs=2, space="PSUM") as ps_tp, \
             tc.tile_pool(name="psout", bufs=2, space="PSUM") as ps_out:

            # ---- on-device constants ----
            crow_t = cpool.tile([1, 128], DT)
            nc.sync.dma_start(out=crow_t[:], in_=crow_d.ap())
            bias_t = cpool.tile([1, ES], mybir.dt.float32)
            nc.sync.dma_start(out=bias_t[:], in_=bias_d.ap())
            Wsb = cpool.tile([32, 4, 32], DT)
            nc.sync.dma_start(out=Wsb[:], in_=W_d.ap())
            ident = cpool.tile([128, 128], DT)
            make_identity(nc, ident[:])
            ones_t = cpool.tile([1, 128], DT)
            nc.vector.memset(ones_t[:], 1.0)
            onesf_t = cpool.tile([1, 128], mybir.dt.float32)
            nc.vector.memset(onesf_t[:], 1.0)

            # iota row broadcast to all 128 partitions via rank-1 matmul:
            # iota_t[p, j] = j
            iota_t = cpool.tile([128, 128], DT)
            psI = ps_tp.tile([128, 128], mybir.dt.float32, tag="tp")
            nc.tensor.matmul(psI[:], lhsT=ones_t[:], rhs=crow_t[:],
                             start=True, stop=True)
            nc.scalar.copy(out=iota_t[:], in_=psI[:])

            # ET[c, p] = (p % 32 == c): four side-by-side 32x32 identities
            ET = cpool.tile([32, 128], DT)
            for j in range(4):
                nc.vector.tensor_copy(out=ET[:, 32 * j:32 * (j + 1)],
                                      in_=ident[:32, :32])

            # block-diagonal W tiles B[a][k]: [128, 384]:
            # B[a,k][p, c'*12 + t] = W_a[p % 32, c'] with t = 4k + p//32
            B_t = cpool.tile([128, 4, 3, ES], DT)
            nc.vector.memset(B_t[:], 0.0)
            for a in range(4):
                psG = ps_tp.tile([128, 32], mybir.dt.float32, tag="tp")
                nc.tensor.matmul(psG[:], lhsT=ET[:], rhs=Wsb[:, a, :],
                                 start=True, stop=True)
                G = cpool.tile([128, 32], DT, name=f"G{a}")
                nc.scalar.copy(out=G[:], in_=psG[:])
                for k in range(3):
                    Bv = B_t[:, a, k, :].rearrange("p (c t) -> p t c", t=T)
                    for j in range(4):
                        nc.vector.tensor_copy(
                            out=Bv[32 * j:32 * (j + 1), 4 * k + j, :],
                            in_=G[32 * j:32 * (j + 1), :])

            # bias broadcast to 128 partitions via fp32 rank-1 matmul
            biasb_t = cpool.tile([128, ES], mybir.dt.float32)
            psB = ps_out.tile([128, ES], mybir.dt.float32, tag="wout")
            nc.tensor.matmul(psB[:], lhsT=onesf_t[:], rhs=bias_t[:],
                             start=True, stop=True)
            nc.scalar.copy(out=biasb_t[:], in_=psB[:])

            max_nch = max(int(v) for ncw in ncws for v in ncw)
            choffs = [[int(sum(ncws[ai][:w])) for w in range(NWIN)]
                      for ai in range(4)]

            for _rep in range(rep):
                # ---- x all-gather: 3750-row shard -> full 30000-row copy ----
                xint = dpool.tile([ND, ES], DT, tag="xint")
                xfull = dpool.tile([N, ES], DT, tag="xfull")
                nc.gpsimd.dma_start(out=xint[:], in_=xsh_d.ap())
                if "noag" in ablate:
                    nc.gpsimd.dma_start(out=xfull[:ND], in_=xint[:])
                else:
                    nc.gpsimd.collective_compute(
                        "AllGather", mybir.AluOpType.bypass,
                        replica_groups=[list(range(NCORES))],
                        ins=[xint[:].opt()], outs=[xfull[:].opt()])

                # ---- resident edge tensors, loaded once per pass ----
                # gather idx: the [16, n] host tensor replicated into all 8
                # partition groups; dst columns: uint8, cast to DT in-DMA
                idx_all = [wpool.tile([128, nchs[ai] * 8], mybir.dt.int16,
                                      tag=f"idx{a}", bufs=1, name=f"idx_{a}")
                           for ai, a in enumerate(ADJS)]
                dlc_all = [wpool.tile([128, nchs[ai]], DT,
                                      tag=f"dlc{a}", bufs=1, name=f"dlc_{a}")
                           for ai, a in enumerate(ADJS)]
                ew_all = [wpool.tile([128, nchs[ai]], DT,
                                     tag=f"ew{a}", bufs=1, name=f"ewt_{a}")
                          for ai, a in enumerate(ADJS)]
                for ai in range(4):
                    for r in range(8):
                        nc.sync.dma_start(out=idx_all[ai][16 * r:16 * (r + 1), :],
                                          in_=gi_d[ai].ap())
                    nc.gpsimd.dma_start(out=dlc_all[ai][:], in_=dlc_d[ai].ap())
                    nc.scalar.dma_start(out=ew_all[ai][:], in_=ew_d[ai].ap())

                # ---- fused aggregation + W transform, window-major ----
                for w in range(NWIN):
                    yws = []
                    for ai in range(4):
                        nch = int(ncws[ai][w])
                        choff = choffs[ai][w]
                        # on-chip banded matrix build (whole window)
                        st_t = wpool.tile([128, max_nch, WIN], DT, tag="st", bufs=2)
                        dlc_t = dlc_all[ai][:, choff:choff + nch]
                        ewc_t = ew_all[ai][:, choff:choff + nch]
                        idx_t = idx_all[ai]
                        if "nostbuild" in ablate:
                            nc.vector.tensor_copy(
                                out=st_t[:, :nch, :],
                                in_=iota_t[:].rearrange("p (o i) -> p o i", o=1)
                                    .to_broadcast([128, nch, WIN]))
                        else:
                            dlc_b = dlc_t.rearrange("p (n o) -> p n o", o=1) \
                                .to_broadcast([128, nch, WIN])
                            iota_b = iota_t[:].rearrange("p (o i) -> p o i", o=1) \
                                .to_broadcast([128, nch, WIN])
                            ewc_b = ewc_t.rearrange("p (n o) -> p n o", o=1) \
                                .to_broadcast([128, nch, WIN])
                            nc.vector.tensor_tensor(out=st_t[:, :nch, :], in0=dlc_b,
                                                    in1=iota_b,
                                                    op=mybir.AluOpType.is_equal)
                            nc.vector.tensor_tensor(out=st_t[:, :nch, :],
                                                    in0=st_t[:, :nch, :], in1=ewc_b,
                                                    op=mybir.AluOpType.mult)
                        ps = ps_agg.tile([128, ES], mybir.dt.float32, tag="agg")
                        # gather in fixed-size blocks for deep DMA pipelining
                        for b0 in range(0, nch, GBLK):
                            bn = min(GBLK, nch - b0)
                            msg = wpool.tile([128, GBLK, ES], DT,
                                             tag="msg", bufs=GBUFS)
                            if "nogather" not in ablate:
                                nc.gpsimd.dma_gather(
                                    msg[:, :bn, :], xfull[:].opt(),
                                    idx_t[:, (choff + b0) * 8:(choff + b0 + bn) * 8],
                                    bn * 128, bn * 128, ES, elem_step=ES,
                                    single_packet=False, queue_num=gq[0] % 4)
                                gq[0] += 1
                            else:
                                nc.gpsimd.memset(msg[:], 0.0)
                            if "noaggmm" in ablate:
                                nc.tensor.matmul(ps[:, :], lhsT=st_t[:, b0, :],
                                                 rhs=msg[:, 0, :],
                                                 start=(b0 == 0), stop=(b0 + bn >= nch))
                            else:
                                for j in range(bn):
                                    ch = b0 + j
                                    nc.tensor.matmul(ps[:, :], lhsT=st_t[:, ch, :],
                                                     rhs=msg[:, j, :],
                                                     start=(ch == 0), stop=(ch == nch - 1))
                        yw = ypool.tile([128, ES], DT, tag="yw", bufs=8,
                                        name=f"yw_{ai}")
                        nc.scalar.copy(out=yw[:], in_=ps[:])
                        yws.append(yw)

                    if "nowphase" in ablate:
                        continue
                    yTs = []
                    for ai in range(4):
                        for k in range(3):
                            pst = ps_tp.tile([128, 128], DT, tag="tp")
                            nc.tensor.transpose(
                                pst[:], yws[ai][:, 128 * k:128 * (k + 1)], ident[:])
                            yT = ytpool.tile([128, 128], DT, tag="yT")
                            eng = nc.vector if (ai * 3 + k) % 2 == 0 else nc.scalar
                            if eng is nc.vector:
                                eng.tensor_copy(out=yT[:], in_=pst[:])
                            else:
                                eng.copy(out=yT[:], in_=pst[:])
                            yTs.append(yT)
                    pso = ps_out.tile([128, ES], mybir.dt.float32, tag="wout")
                    for i, yT in enumerate(yTs):
                        ai, k = divmod(i, 3)
                        nc.tensor.matmul(pso[:], lhsT=yT[:], rhs=B_t[:, ai, k, :],
                                         start=(i == 0), stop=(i == 11))
                    outsb = opool.tile([128, ES], DT, tag="out")
                    nc.vector.tensor_tensor(out=outsb[:], in0=pso[:], in1=biasb_t[:],
                                            op=mybir.AluOpType.add)
                    nr = min(128, ND - 128 * w)
                    nc.sync.dma_start(out=out_d.ap()[128 * w:128 * w + nr, :],
                                      in_=outsb[:nr])

    nc.compile()
    _CACHE[key] = nc
    return nc


def _host_prep(x, Ws, bias, eis, ews):
    x_rows = np.ascontiguousarray(
        np.asarray(x).astype(np.float32).transpose(0, 2, 1).reshape(N, ES)).astype(bf16)
    ncws, gis, dlcs = [], [], []
    for a in ADJS:
        pc = _prep_edges(np.asarray(eis[a]), np.asarray(ews[a]))
        ncw, gi, dew = _build_adj_inputs(pc)
        ncws.append(tuple(int(v) for v in ncw))
        gis.append(gi)
        dlcs.append(dew)
    Wmats = np.ascontiguousarray(
        np.stack([np.asarray(w).astype(np.float32) for w in Ws], 1)).astype(bf16)
    biasrow = np.ascontiguousarray(
        np.repeat(np.asarray(bias).astype(np.float32), T)[None, :])
    crows = np.ascontiguousarray(
        np.arange(128, dtype=np.float32)[None, :]).astype(bf16)
    in_maps = []
    for k in range(NCORES):
        m = {"xsh": np.ascontiguousarray(x_rows[k * ND:(k + 1) * ND]),
             "Wmats": Wmats, "biasrow": biasrow, "crows": crows}
        for ai, a in enumerate(ADJS):
            m[f"gi_{a}"] = gis[ai][k]
            m[f"dlc_{a}"] = dlcs[ai][k][0]
            m[f"ew_{a}"] = dlcs[ai][k][1]
        in_maps.append(m)
    return tuple(ncws), in_maps


def kernel(x, W_fwd1, W_fwd2, W_bck1, W_bck2, bias,
           ew_fwd1, ew_fwd2, ew_bck1, ew_bck2,
           ei_fwd1, ei_fwd2, ei_bck1, ei_bck2):
    from concourse.bass_utils import run_bass_kernel_spmd

    x = np.asarray(x)
    eis = dict(fwd1=ei_fwd1, fwd2=ei_fwd2, bck1=ei_bck1, bck2=ei_bck2)
    ews = dict(fwd1=ew_fwd1, fwd2=ew_fwd2, bck1=ew_bck1, bck2=ew_bck2)
    Ws = [W_fwd1, W_fwd2, W_bck1, W_bck2]

    ncws, in_maps = _host_prep(x, Ws, bias, eis, ews)
    nc = _get_program(ncws)

    res = run_bass_kernel_spmd(nc, in_maps, core_ids=list(range(NCORES)))
    global LAST_RESULTS, LAST_NC, LAST_INMAPS, LAST_NCWS
    LAST_RESULTS = res
    LAST_NC = nc
    LAST_INMAPS = in_maps
    LAST_NCWS = ncws

    out = np.empty((N, C, T), np.float32)
    for k in range(NCORES):
        shard = res.results[k]["out"]                # [3750, 384], phi'=c*12+t
        out[k * ND:(k + 1) * ND] = shard.astype(np.float32).reshape(ND, C, T)
    return out
